# revision 8
# baseline (speedup 1.0000x reference)
"""LSTMCell kernel for Trainium2, data-parallel over 8 NeuronCores.

Reference op (B=32768, DIN=512, H=512, fp32):
    xc = concat([h, x], -1)                       # [B, 1024]
    f,i,o = sigmoid(xc @ W{f,i,o}.T + b{f,i,o});  g = tanh(xc @ Wg.T + bg)
    c_new = c*f + i*g;  h_new = o*tanh(c_new)

Strategy: shard batch across 8 cores (4096 rows each). Per core, process
32 tiles of 128 rows. Matmuls run in bf16 (fp32 PSUM accumulate): the
stationary operand is the transposed activation tile xcT [K=128, B=128]
(pre-transposed/tiled on host), the moving operand is W.T [K=128, H=512]
(resident in SBUF). Output lands naturally as [B=128, H=512] so the cell
update needs no on-chip transposes. Bias is added on DVE (PSUM->SBUF),
activations on ACT, elementwise on DVE.
"""

import numpy as np
import ml_dtypes

import concourse.bass as bass
import concourse.mybir as mybir
import concourse.tile as tile
from concourse import bacc
from concourse.bass_utils import run_bass_kernel_spmd

B, DIN, H = 32768, 512, 512
NCORES = 8
BS = B // NCORES          # 4096 rows per core
T = 128                   # rows per tile
NT = BS // T              # 32 tiles per core
NTG = B // T              # 256 tiles global
KC = (DIN + H) // 128     # 8 K-chunks

BF16 = ml_dtypes.bfloat16

# MM_DT: 'bf16' (fast) or 'f32' (4x slower PE, full precision)
MM_DT = "bf16"

_nc_cache = {}


def _build_nc():
    key = MM_DT
    if key in _nc_cache:
        return _nc_cache[key]
    mdt = mybir.dt.bfloat16 if MM_DT == "bf16" else mybir.dt.float32
    f32 = mybir.dt.float32
    AF = mybir.ActivationFunctionType

    nc = bacc.Bacc()
    xcT = nc.dram_tensor("xcT", [NT, 128, KC, 128], mdt, kind="ExternalInput")
    wT = nc.dram_tensor("wT", [128, 4, KC, 512], mdt, kind="ExternalInput")
    bias = nc.dram_tensor("bias", [1, 4, 512], mdt, kind="ExternalInput")
    c_in = nc.dram_tensor("c_in", [NT, 128, 512], f32, kind="ExternalInput")
    h_out = nc.dram_tensor("h_out", [NT, 128, 512], f32, kind="ExternalOutput")
    c_out = nc.dram_tensor("c_out", [NT, 128, 512], f32, kind="ExternalOutput")

    GATE_FUNC = [AF.Sigmoid, AF.Sigmoid, AF.Tanh, AF.Sigmoid]  # f, i, g, o

    with tile.TileContext(nc) as tc:
        with (
            tc.tile_pool(name="const", bufs=1) as cpool,
            tc.tile_pool(name="io", bufs=4) as iopool,
            tc.tile_pool(name="work", bufs=3) as wpool,
            tc.tile_pool(name="psum", bufs=2, space=bass.MemorySpace.PSUM) as pspool,
        ):
            wt_sb = cpool.tile([128, 4, KC, 512], mdt)
            nc.sync.dma_start(wt_sb[:], wT[:])
            bias_sb = cpool.tile([1, 4, 512], mdt)
            nc.sync.dma_start(bias_sb[:], bias[:])
            ones_sb = cpool.tile([1, 128], mdt)
            nc.gpsimd.memset(ones_sb[:], 1.0)

            for t in range(NT):
                xct = iopool.tile([128, KC, 128], mdt, tag="xct")
                nc.sync.dma_start(xct[:], xcT[t])
                c_t = iopool.tile([128, 512], f32, tag="c")
                nc.sync.dma_start(c_t[:], c_in[t])

                ps = pspool.tile([128, 4, 512], f32, tag="ps")
                # bias as a rank-1 first matmul: ones[1,128].T @ b[1,512]
                for g in range(4):
                    nc.tensor.matmul(
                        ps[:, g, :], ones_sb[:], bias_sb[:, g, :],
                        start=True, stop=False,
                    )
                for k in range(KC):
                    for g in range(4):
                        nc.tensor.matmul(
                            ps[:, g, :],
                            xct[:, k, :],
                            wt_sb[:, g, k, :],
                            start=False,
                            stop=(k == KC - 1),
                        )

                acts = []
                for g in range(4):
                    a = wpool.tile([128, 512], f32, tag=f"act{g}")
                    nc.scalar.activation(a[:], ps[:, g, :], GATE_FUNC[g])
                    acts.append(a)
                f_a, i_a, g_a, o_a = acts

                cf = wpool.tile([128, 512], f32, tag="cf")
                nc.vector.tensor_mul(cf[:], c_t[:], f_a[:])
                ig = wpool.tile([128, 512], f32, tag="ig")
                nc.vector.tensor_mul(ig[:], i_a[:], g_a[:])
                cn = wpool.tile([128, 512], f32, tag="cn")
                nc.vector.tensor_add(cn[:], cf[:], ig[:])
                tc_t = wpool.tile([128, 512], f32, tag="tc")
                nc.scalar.activation(tc_t[:], cn[:], AF.Tanh)
                hn = wpool.tile([128, 512], f32, tag="hn")
                nc.vector.tensor_mul(hn[:], o_a[:], tc_t[:])

                nc.sync.dma_start(c_out[t], cn[:])
                nc.sync.dma_start(h_out[t], hn[:])

    nc.compile()
    _nc_cache[key] = nc
    return nc


def _host_prep(x, h, c, Wf, bf, Wi, bi, Wo, bo, Wg, bg):
    npdt = BF16 if MM_DT == "bf16" else np.float32
    # xcT tiles: [tile, kk(partition), kchunk, b] ; chunks 0-3 = h, 4-7 = x
    xcT = np.empty((NTG, 128, KC, 128), dtype=npdt)
    xcT[:, :, 0:4, :] = h.reshape(NTG, 128, 4, 128).transpose(0, 3, 2, 1)
    xcT[:, :, 4:8, :] = x.reshape(NTG, 128, 4, 128).transpose(0, 3, 2, 1)

    Wall = np.stack([Wf, Wi, Wg, Wo])  # [4, H, K] ; gate order f,i,g,o
    wT = np.ascontiguousarray(
        Wall.reshape(4, 512, KC, 128).transpose(3, 0, 2, 1)
    ).astype(npdt)  # [kk, gate, kchunk, h]

    bias = np.stack([bf, bi, bg, bo]).astype(npdt)[None, :, :]  # [1, 4, 512]

    c_resh = np.ascontiguousarray(c.reshape(NTG, 128, 512)).astype(np.float32)
    return xcT, wT, bias, c_resh


def kernel(x, h, c, Wf, bf, Wi, bi, Wo, bo, Wg, bg, trace=False, tmpdir=None):
    x = np.asarray(x, dtype=np.float32)
    h = np.asarray(h, dtype=np.float32)
    c = np.asarray(c, dtype=np.float32)
    xcT, wT, bias, c_resh = _host_prep(
        x, h, c,
        np.asarray(Wf), np.asarray(bf), np.asarray(Wi), np.asarray(bi),
        np.asarray(Wo), np.asarray(bo), np.asarray(Wg), np.asarray(bg),
    )

    nc = _build_nc()
    in_maps = []
    for core in range(NCORES):
        sl = slice(core * NT, (core + 1) * NT)
        in_maps.append({
            "xcT": np.ascontiguousarray(xcT[sl]),
            "wT": wT,
            "bias": bias,
            "c_in": np.ascontiguousarray(c_resh[sl]),
        })
    kernel.last_in_maps = in_maps
    res = run_bass_kernel_spmd(
        nc, in_maps, core_ids=list(range(NCORES)), trace=trace, tmpdir=tmpdir
    )
    h_new = np.concatenate(
        [r["h_out"].reshape(BS, 512) for r in res.results], axis=0
    )
    c_new = np.concatenate(
        [r["c_out"].reshape(BS, 512) for r in res.results], axis=0
    )
    kernel.last_results = res
    return h_new, c_new
